# revision 6
# baseline (speedup 1.0000x reference)
"""Trainium2 Bass kernel for nn_AttentionNetwork (temporal attention pooling).

Reference computation (B=4, F=256, T=8192, H=1024, C=128):
    z         = einsum("bft,fh->bth", seq, Wb) + bb          [B,T,H]
    logits    = z @ Wa + ba                                   [B,T,C]
    attention = softmax(logits, axis=2) / T                   [B,T,C]
    rep       = einsum("bth,btc->bhc", z, attention)          [B,H,C]
    action    = einsum("bhc,hc->bc", rep, A) + action_bias    [B,C]
    thres     = (rep.transpose(0,2,1) @ Wt)[...,0] + bt       [B,C]

Sharding: 8 cores = 4 batch x 2 T-halves (T_loc = 4096 per core).
Each core computes its attention slice plus partial (T-half) sums of
rep / action / thres; the host sums the two partials per batch element
and adds the biases (the epilogue is linear in rep).

Key algebraic refactor: logits = seq.T @ (Wb@Wa) + (bb@Wa + ba), so the
logits matmul contracts over F=256 in the same [t,*] orientation as z and
z never needs a transposed copy.

Matmuls run as float32r (fp32 stored, fp22 multiply, fp32 accumulate) --
4x the fp32 matmul throughput on the PE array.
"""

import numpy as np

import concourse.bacc as bacc
import concourse.bass as bass  # noqa: F401  (AP helpers)
import concourse.mybir as mybir
import concourse.tile as tile
from concourse.bass_utils import run_bass_kernel_spmd

B, F, T, H, C = 4, 256, 8192, 1024, 128
NCORES = 8
TSPLIT = NCORES // B          # 2 T-shards per batch element
TLOC = T // TSPLIT            # 4096 timesteps per core
PT = 128                      # t-tile (partition dim)
NT = TLOC // PT               # 32 t-tiles
FK = F // 128                 # 2 contraction tiles over F
HB = 512                      # h-chunk per matmul (one PSUM bank, fp32)
NSEQ_CHUNKS = 8               # DMA pipelining chunks for the seq load

F32 = mybir.dt.float32
F32R = mybir.dt.float32r      # fp22 multiply / fp32 accumulate on PE


def _r(ap):
    """View an fp32 AP as float32r for the tensor engine."""
    return ap.bitcast(F32R)


def build_nc():
    nc = bacc.Bacc(trn_type="TRN2")

    # Per-core inputs (host pre-shards / pre-broadcasts).
    seq_s = nc.dram_tensor("seq_s", [F, TLOC], F32R, kind="ExternalInput")
    wb = nc.dram_tensor("wb", [F, H], F32R, kind="ExternalInput")
    wf = nc.dram_tensor("wf", [F, C], F32R, kind="ExternalInput")
    bb_bc = nc.dram_tensor("bb_bc", [128, H], F32, kind="ExternalInput")
    bf_bc = nc.dram_tensor("bf_bc", [128, C], F32, kind="ExternalInput")
    at_t = nc.dram_tensor("at_t", [C, H], F32, kind="ExternalInput")
    wt_bc = nc.dram_tensor("wt_bc", [C, H], F32, kind="ExternalInput")

    att_out = nc.dram_tensor("att_out", [TLOC, C], F32R, kind="ExternalOutput")
    rep_out = nc.dram_tensor("rep_out", [C, H], F32, kind="ExternalOutput")
    act_out = nc.dram_tensor("act_out", [C, 1], F32, kind="ExternalOutput")
    thr_out = nc.dram_tensor("thr_out", [C, 1], F32, kind="ExternalOutput")

    with tile.TileContext(nc) as tc:
        with (
            tc.tile_pool(name="consts", bufs=1) as consts,
            tc.tile_pool(name="zpool", bufs=3) as zpool,
            tc.tile_pool(name="small", bufs=4) as small,
            tc.tile_pool(name="psz", bufs=2, space="PSUM") as psz,
            tc.tile_pool(name="pslg", bufs=2, space="PSUM") as pslg,
            tc.tile_pool(name="psrep", bufs=1, space="PSUM") as psrep,
        ):
            # ---- constant loads -------------------------------------------
            wb_sb = consts.tile([128, FK, H], F32R)
            nc.sync.dma_start(out=wb_sb, in_=wb.rearrange("(k p) h -> p k h", p=128))
            wf_sb = consts.tile([128, FK, C], F32R)
            nc.sync.dma_start(out=wf_sb, in_=wf.rearrange("(k p) c -> p k c", p=128))
            bb_sb = consts.tile([128, H], F32)
            nc.sync.dma_start(out=bb_sb, in_=bb_bc[:, :])
            bf_sb = consts.tile([128, C], F32)
            nc.sync.dma_start(out=bf_sb, in_=bf_bc[:, :])
            at_sb = consts.tile([C, H], F32)
            nc.sync.dma_start(out=at_sb, in_=at_t[:, :])
            wt_sb = consts.tile([C, H], F32)
            nc.sync.dma_start(out=wt_sb, in_=wt_bc[:, :])

            # seq resident in SBUF, loaded in chunks so compute can start early
            seq_sb = consts.tile([128, FK, TLOC], F32R)
            seq_src = seq_s.rearrange("(k p) t -> p k t", p=128)
            tchunk = TLOC // NSEQ_CHUNKS
            for ci in range(NSEQ_CHUNKS):
                sl = slice(ci * tchunk, (ci + 1) * tchunk)
                nc.sync.dma_start(out=seq_sb[:, :, sl], in_=seq_src[:, :, sl])

            # rep accumulator lives in PSUM across the whole t-loop
            ps_rep = psrep.tile([C, H], F32)

            # ---- main loop over 32 t-tiles --------------------------------
            for i in range(NT):
                ts = slice(i * PT, (i + 1) * PT)

                ps_z = psz.tile([PT, H], F32)
                ps_lg = pslg.tile([PT, C], F32)
                for k in range(FK):
                    lhs = seq_sb[:, k, ts]
                    st, sp = (k == 0), (k == FK - 1)
                    for hb in range(H // HB):
                        hs = slice(hb * HB, (hb + 1) * HB)
                        nc.tensor.matmul(
                            ps_z[:, hs], lhs, wb_sb[:, k, hs], start=st, stop=sp
                        )
                    nc.tensor.matmul(
                        ps_lg, lhs, wf_sb[:, k, :], start=st, stop=sp
                    )

                # z = psum + bb  (evacuate to SBUF with the bias add)
                z_sb = zpool.tile([PT, H], F32R)
                for hb in range(H // HB):
                    hs = slice(hb * HB, (hb + 1) * HB)
                    nc.vector.tensor_add(z_sb[:, hs], ps_z[:, hs], bb_sb[:, hs])

                # softmax over classes (free dim) / T
                lg = small.tile([PT, C], F32)
                nc.vector.tensor_add(lg, ps_lg, bf_sb)
                e = small.tile([PT, C], F32)
                ssum = small.tile([PT, 1], F32)
                nc.scalar.activation(
                    e, lg, mybir.ActivationFunctionType.Exp, accum_out=ssum
                )
                rcp = small.tile([PT, 1], F32)
                nc.vector.reciprocal(rcp, ssum)
                att = small.tile([PT, C], F32R)
                nc.vector.tensor_scalar(
                    att, e, rcp, 1.0 / T,
                    mybir.AluOpType.mult, mybir.AluOpType.mult,
                )
                nc.sync.dma_start(out=att_out[ts, :], in_=att)

                # rep^T[c,h] += att[t,c].T @ z[t,h]
                for hb in range(H // HB):
                    hs = slice(hb * HB, (hb + 1) * HB)
                    nc.tensor.matmul(
                        ps_rep[:, hs], att, z_sb[:, hs],
                        start=(i == 0), stop=(i == NT - 1),
                    )

            # ---- epilogue: rep out + partial action/thres -----------------
            rep_sb = consts.tile([C, H], F32)
            nc.scalar.copy(rep_sb, ps_rep)
            nc.sync.dma_start(out=rep_out[:, :], in_=rep_sb)

            tmp_a = consts.tile([C, H], F32)
            act_acc = consts.tile([C, 1], F32)
            nc.vector.tensor_mul(tmp_a, rep_sb, at_sb)
            nc.vector.reduce_sum(act_acc, tmp_a, axis=mybir.AxisListType.X)
            nc.sync.dma_start(out=act_out[:, :], in_=act_acc)

            tmp_t = consts.tile([C, H], F32)
            thr_acc = consts.tile([C, 1], F32)
            nc.vector.tensor_mul(tmp_t, rep_sb, wt_sb)
            nc.vector.reduce_sum(thr_acc, tmp_t, axis=mybir.AxisListType.X)
            nc.sync.dma_start(out=thr_out[:, :], in_=thr_acc)

    nc.finalize()
    return nc


def _prepare_in_maps(seq, Wb, bb, Wa, ba, action_matrix, Wt):
    seq = np.ascontiguousarray(np.asarray(seq, dtype=np.float32))
    Wb = np.ascontiguousarray(np.asarray(Wb, dtype=np.float32))
    bb = np.asarray(bb, dtype=np.float32)
    Wa = np.asarray(Wa, dtype=np.float32)
    ba = np.asarray(ba, dtype=np.float32)
    A = np.asarray(action_matrix, dtype=np.float32)
    Wt = np.asarray(Wt, dtype=np.float32)

    wf = (Wb.astype(np.float64) @ Wa.astype(np.float64)).astype(np.float32)
    bf = (bb.astype(np.float64) @ Wa.astype(np.float64)
          + ba.astype(np.float64)).astype(np.float32)

    bb_bc = np.ascontiguousarray(np.broadcast_to(bb[None, :], (128, H)))
    bf_bc = np.ascontiguousarray(np.broadcast_to(bf[None, :], (128, C)))
    at_t = np.ascontiguousarray(A.T)                      # [C, H]
    wt_bc = np.ascontiguousarray(np.broadcast_to(Wt[:, 0][None, :], (C, H)))

    in_maps = []
    for core in range(NCORES):
        b, sh = core // TSPLIT, core % TSPLIT
        t0 = sh * TLOC
        in_maps.append({
            "seq_s": np.ascontiguousarray(seq[b, :, t0:t0 + TLOC]),
            "wb": Wb, "wf": wf, "bb_bc": bb_bc, "bf_bc": bf_bc,
            "at_t": at_t, "wt_bc": wt_bc,
        })
    return in_maps


def _assemble(results, action_bias, bt):
    action_bias = np.asarray(action_bias, dtype=np.float32)
    bt = np.asarray(bt, dtype=np.float32)

    attention = np.empty((B, T, C), dtype=np.float32)
    rep_t = np.zeros((B, C, H), dtype=np.float32)
    act = np.zeros((B, C), dtype=np.float32)
    thr = np.zeros((B, C), dtype=np.float32)
    for core in range(NCORES):
        r = results[core]
        b, sh = core // TSPLIT, core % TSPLIT
        t0 = sh * TLOC
        attention[b, t0:t0 + TLOC, :] = r["att_out"]
        rep_t[b] += r["rep_out"]
        act[b] += r["act_out"][:, 0]
        thr[b] += r["thr_out"][:, 0]

    rep_feature = np.ascontiguousarray(rep_t.transpose(0, 2, 1))  # [B,H,C]
    action_logit = act + action_bias          # action_bias [1,C] broadcasts
    thres = thr + bt
    return attention, rep_feature, action_logit, thres


def run(inputs, **spmd_kwargs):
    """Build, run on 8 cores, and assemble. Returns (outputs, BassKernelResults)."""
    nc = build_nc()
    in_maps = _prepare_in_maps(
        inputs["seq"], inputs["Wb"], inputs["bb"], inputs["Wa"], inputs["ba"],
        inputs["action_matrix"], inputs["Wt"],
    )
    res = run_bass_kernel_spmd(nc, in_maps, core_ids=list(range(NCORES)),
                               **spmd_kwargs)
    outs = _assemble(res.results, inputs["action_bias"], inputs["bt"])
    return outs, res


def kernel(**inputs):
    outs, _ = run(inputs)
    return outs


# revision 7
# speedup vs baseline: 1.2578x; 1.2578x over previous
"""Trainium2 Bass kernel for nn_AttentionNetwork (temporal attention pooling).

Reference computation (B=4, F=256, T=8192, H=1024, C=128):
    z         = einsum("bft,fh->bth", seq, Wb) + bb          [B,T,H]
    logits    = z @ Wa + ba                                   [B,T,C]
    attention = softmax(logits, axis=2) / T                   [B,T,C]
    rep       = einsum("bth,btc->bhc", z, attention)          [B,H,C]
    action    = einsum("bhc,hc->bc", rep, A) + action_bias    [B,C]
    thres     = (rep.transpose(0,2,1) @ Wt)[...,0] + bt       [B,C]

Sharding: 8 cores = 4 batch x 2 T-halves (T_loc = 4096 per core).
Each core computes its attention slice and the partial (T-half)
rep0^T = att^T @ z0 where z0 = seq^T @ Wb (no bias). The host sums the
two partials, applies the rank-1 bias correction
rep += outer(sum_t att, bb), and runs the tiny linear epilogue
(action/thres) -- everything downstream of rep is linear in it.

Algebraic refactors keeping all device matmuls in the [t,*] orientation:
  logits = seq^T @ (Wb@Wa) + (bb@Wa + ba)   (fused on host)
  rep    = att^T @ z0 + outer(sum_t att, bb) (corrected on host)

Matmuls run as float32r (fp32 stored, fp22 multiply, fp32 accumulate) --
4x the fp32 matmul rate on the PE array.
"""

import numpy as np

import concourse.bacc as bacc
import concourse.mybir as mybir
import concourse.tile as tile
from concourse.bass_utils import run_bass_kernel_spmd

B, F, T, H, C = 4, 256, 8192, 1024, 128
NCORES = 8
TSPLIT = NCORES // B          # 2 T-shards per batch element
TLOC = T // TSPLIT            # 4096 timesteps per core
PT = 128                      # t-tile (partition dim)
NT = TLOC // PT               # 32 t-tiles
FK = F // 128                 # 2 contraction tiles over F
HB = 512                      # h-chunk per matmul (one PSUM bank, fp32)
NSEQ_CHUNKS = 8               # DMA pipelining chunks for the seq load

F32 = mybir.dt.float32
F32R = mybir.dt.float32r      # fp22 multiply / fp32 accumulate on PE


def build_nc():
    nc = bacc.Bacc(trn_type="TRN2")

    # Per-core inputs (host pre-shards / pre-broadcasts).
    seq_s = nc.dram_tensor("seq_s", [F, TLOC], F32R, kind="ExternalInput")
    wb = nc.dram_tensor("wb", [F, H], F32R, kind="ExternalInput")
    wf = nc.dram_tensor("wf", [F, C], F32R, kind="ExternalInput")
    bf_bc = nc.dram_tensor("bf_bc", [128, C], F32, kind="ExternalInput")

    att_out = nc.dram_tensor("att_out", [TLOC, C], F32R, kind="ExternalOutput")
    rep_out = nc.dram_tensor("rep_out", [C, H], F32, kind="ExternalOutput")

    with tile.TileContext(nc) as tc:
        with (
            tc.tile_pool(name="consts", bufs=1) as consts,
            tc.tile_pool(name="zpool", bufs=3) as zpool,
            tc.tile_pool(name="small", bufs=4) as small,
            tc.tile_pool(name="psz", bufs=2, space="PSUM") as psz,
            tc.tile_pool(name="pslg", bufs=2, space="PSUM") as pslg,
            tc.tile_pool(name="psrep", bufs=1, space="PSUM") as psrep,
        ):
            # ---- constant loads -------------------------------------------
            wb_sb = consts.tile([128, FK, H], F32R)
            nc.sync.dma_start(out=wb_sb, in_=wb.rearrange("(k p) h -> p k h", p=128))
            wf_sb = consts.tile([128, FK, C], F32R)
            nc.sync.dma_start(out=wf_sb, in_=wf.rearrange("(k p) c -> p k c", p=128))
            bf_sb = consts.tile([128, C], F32)
            nc.sync.dma_start(out=bf_sb, in_=bf_bc[:, :])

            # seq resident in SBUF, loaded in chunks so compute can start early
            seq_sb = consts.tile([128, FK, TLOC], F32R)
            seq_src = seq_s.rearrange("(k p) t -> p k t", p=128)
            tchunk = TLOC // NSEQ_CHUNKS
            for ci in range(NSEQ_CHUNKS):
                sl = slice(ci * tchunk, (ci + 1) * tchunk)
                nc.sync.dma_start(out=seq_sb[:, :, sl], in_=seq_src[:, :, sl])

            # rep accumulator lives in PSUM across the whole t-loop
            ps_rep = psrep.tile([C, H], F32)

            # ---- main loop over 32 t-tiles --------------------------------
            for i in range(NT):
                ts = slice(i * PT, (i + 1) * PT)

                ps_z = psz.tile([PT, H], F32)
                ps_lg = pslg.tile([PT, C], F32)
                for k in range(FK):
                    lhs = seq_sb[:, k, ts]
                    st, sp = (k == 0), (k == FK - 1)
                    for hb in range(H // HB):
                        hs = slice(hb * HB, (hb + 1) * HB)
                        nc.tensor.matmul(
                            ps_z[:, hs], lhs, wb_sb[:, k, hs], start=st, stop=sp
                        )
                    nc.tensor.matmul(
                        ps_lg, lhs, wf_sb[:, k, :], start=st, stop=sp
                    )

                # evacuate z (no bias needed): split between ACT and DVE
                z_sb = zpool.tile([PT, H], F32R)
                nc.scalar.copy(z_sb[:, 0:HB], ps_z[:, 0:HB])
                nc.vector.tensor_copy(z_sb[:, HB:H], ps_z[:, HB:H])

                # softmax over classes (free dim) / T
                lg = small.tile([PT, C], F32)
                nc.vector.tensor_add(lg, ps_lg, bf_sb)
                e = small.tile([PT, C], F32)
                ssum = small.tile([PT, 1], F32)
                nc.scalar.activation(
                    e, lg, mybir.ActivationFunctionType.Exp, accum_out=ssum
                )
                rcp = small.tile([PT, 1], F32)
                nc.vector.reciprocal(rcp, ssum)
                att = small.tile([PT, C], F32R)
                nc.vector.tensor_scalar(
                    att, e, rcp, 1.0 / T,
                    mybir.AluOpType.mult, mybir.AluOpType.mult,
                )
                nc.scalar.dma_start(out=att_out[ts, :], in_=att)

                # rep0^T[c,h] += att[t,c].T @ z0[t,h]
                for hb in range(H // HB):
                    hs = slice(hb * HB, (hb + 1) * HB)
                    nc.tensor.matmul(
                        ps_rep[:, hs], att, z_sb[:, hs],
                        start=(i == 0), stop=(i == NT - 1),
                    )

            # ---- epilogue: partial rep out --------------------------------
            rep_sb = consts.tile([C, H], F32)
            nc.scalar.copy(rep_sb, ps_rep)
            nc.sync.dma_start(out=rep_out[:, :], in_=rep_sb)

    nc.finalize()
    return nc


def _prepare_in_maps(seq, Wb, bb, Wa, ba):
    seq = np.ascontiguousarray(np.asarray(seq, dtype=np.float32))
    Wb = np.ascontiguousarray(np.asarray(Wb, dtype=np.float32))
    bb = np.asarray(bb, dtype=np.float32)
    Wa = np.asarray(Wa, dtype=np.float32)
    ba = np.asarray(ba, dtype=np.float32)

    wf = (Wb.astype(np.float64) @ Wa.astype(np.float64)).astype(np.float32)
    bf = (bb.astype(np.float64) @ Wa.astype(np.float64)
          + ba.astype(np.float64)).astype(np.float32)
    bf_bc = np.ascontiguousarray(np.broadcast_to(bf[None, :], (128, C)))

    in_maps = []
    for core in range(NCORES):
        b, sh = core // TSPLIT, core % TSPLIT
        t0 = sh * TLOC
        in_maps.append({
            "seq_s": np.ascontiguousarray(seq[b, :, t0:t0 + TLOC]),
            "wb": Wb, "wf": wf, "bf_bc": bf_bc,
        })
    return in_maps


def _assemble(results, bb, action_matrix, action_bias, Wt, bt):
    bb = np.asarray(bb, dtype=np.float64)
    A = np.asarray(action_matrix, dtype=np.float64)
    action_bias = np.asarray(action_bias, dtype=np.float64)
    Wt = np.asarray(Wt, dtype=np.float64)
    bt = np.asarray(bt, dtype=np.float64)

    attention = np.empty((B, T, C), dtype=np.float32)
    rep_t = np.zeros((B, C, H), dtype=np.float64)
    for core in range(NCORES):
        r = results[core]
        b, sh = core // TSPLIT, core % TSPLIT
        t0 = sh * TLOC
        attention[b, t0:t0 + TLOC, :] = r["att_out"]
        rep_t[b] += r["rep_out"]

    # rank-1 bias correction: rep^T[c,h] += (sum_t att[t,c]) * bb[h]
    s_att = attention.astype(np.float64).sum(axis=1)          # [B, C]
    rep_t += s_att[:, :, None] * bb[None, None, :]

    rep_feature = np.ascontiguousarray(
        rep_t.transpose(0, 2, 1)).astype(np.float32)          # [B, H, C]
    rep64 = rep_feature.astype(np.float64)
    action_logit = (np.einsum("bhc,hc->bc", rep64, A)
                    + action_bias).astype(np.float32)
    thres = (np.einsum("bhc,h->bc", rep64, Wt[:, 0]) + bt).astype(np.float32)
    return attention, rep_feature, action_logit, thres


def run(inputs, **spmd_kwargs):
    """Build, run on 8 cores, and assemble. Returns (outputs, BassKernelResults)."""
    nc = build_nc()
    in_maps = _prepare_in_maps(
        inputs["seq"], inputs["Wb"], inputs["bb"], inputs["Wa"], inputs["ba"],
    )
    res = run_bass_kernel_spmd(nc, in_maps, core_ids=list(range(NCORES)),
                               **spmd_kwargs)
    outs = _assemble(res.results, inputs["bb"], inputs["action_matrix"],
                     inputs["action_bias"], inputs["Wt"], inputs["bt"])
    return outs, res


def kernel(**inputs):
    outs, _ = run(inputs)
    return outs
